# revision 11
# baseline (speedup 1.0000x reference)
"""Trainium2 Bass kernel for ExpertsChooseMaskedExpand MoE routing.

Math (reference):
    xd[b,e,c,i] = sum_t x[b,t,(e,i)] * dmask[b,t,e,c]            (dispatch)
    y[b,e,c,o]  = sum_i xd[b,e,c,i] * w[e,o,i] + bias[o]         (expert mm)
    out[b,t,o]  = sum_{e,c} y[b,e,c,o] * cmb[b,t,e,c]            (combine)

Restructured (combine applied before the weight matmul — 155 GF total
instead of 215 GF; the E expert matmuls fuse into one K=2048 matmul):
    xd[b,e][c,j] = sum_t dmask[b,e][t,c] * xr[b,e][t,j]
    zT[b,e][j,t] = sum_c xd[b,e][c,j] * cmbT[b,e][c,t]
    out[b][t,o]  = sum_{(e,j)} zT[b][(e,j),t] * wstack[(e,j),o] + s[b][t]*bias[o]
    where s[b][t] = sum_{e,c} cmb[b,t,e,c],  wstack[(e,j),o] = w[e,o,j]

Sharding: 8 cores = (batch b in 0..3) x (expert pair h in 0..1). Core
(b, h) runs dispatch+combine for experts {2h, 2h+1} only (phases 1-2,
K = 1024 of the fused contraction) and produces a partial output over
the FULL 8192 output columns; the host sums the two partials per batch
(fp32) and unpacks the o-major packing. No on-device collectives. The
bias rank-1 term s[t]*bias[o] is applied only on h=0 cores (h=1 cores
receive a zero biasT), fused into the PSUM->SBUF eviction on the
vector engine.

v2: all-bf16 datapath (bf16 matmuls are ~5% faster than float32r and
halve DMA traffic), fp16 output partials, junk-matmul warmup to ramp
the PE p-state during the DMA cold-start window, dedicated input tiles
(no pool-rotation DMA stalls), input DMA ordered so phase 2 is fed the
moment phase 1 drains.

Phase 3 runs transposed: stationary = weight block (j, o-tile), moving
= zT t-chunks, PSUM holds out^T (o, t).
"""

import numpy as np
import ml_dtypes

B, T, E, C = 4, 1024, 4, 512
IN, OUT = 2048, 8192
P = 128
TT = T // P          # 8  t-tiles
CT = C // P          # 4  c-tiles per expert
JT = 4               # j-tiles per expert (i = 512)
EL = 2               # experts handled per core (expert-pair split)
KT = EL * JT         # 8 k-tiles for the fused matmul (K = 1024 per core)
OT = OUT // P        # 64 o-tiles of 128 (full output width per core)
TCH = 2              # t-chunks of 512

_CACHE = {}


def _build_nc():
    import concourse.mybir as mybir
    import concourse.tile as tile
    from concourse import bacc

    f32 = mybir.dt.float32
    bf16 = mybir.dt.bfloat16
    f16 = mybir.dt.float16

    nc = bacc.Bacc("TRN2", target_bir_lowering=False, debug=False, num_devices=8)
    # x_pk[p, e, tt, j] = x[tt*128+p, e*512+j]
    x_t = nc.dram_tensor("x", (P, EL, TT, 512), bf16, kind="ExternalInput")
    # dm_pk[p, e, tt, c] = dm[tt*128+p, e, c]
    dm_t = nc.dram_tensor("dm", (P, EL, TT, C), bf16, kind="ExternalInput")
    # cT_pk[p, e, ct, t] = cmbT[e, ct*128+p, t]
    cT_t = nc.dram_tensor("cmbT", (P, EL, CT, T), bf16, kind="ExternalInput")
    # wpk[p, ot, kt, oi] = wstack[h*1024 + kt*128+p, ot*128 + oi]
    wpk_t = nc.dram_tensor("wpk", (P, OT, KT, P), bf16, kind="ExternalInput")
    sb_t = nc.dram_tensor("sb", (P, T), bf16, kind="ExternalInput")      # s bcast
    bT_t = nc.dram_tensor("biasT", (P, OT), f32, kind="ExternalInput")
    # out_pk[p, ot, tch, u] = out[tch*512+u, ot*128+p]
    o_t = nc.dram_tensor("out", (P, OT, TCH, 512), f16, kind="ExternalOutput")

    x_r = x_t.ap()                                             # [128, 2, 8, 512]
    dm_r = dm_t.ap()                                           # [128, 2, 8, 512]
    cT_r = cT_t.ap()                                           # [128, 2, 4, 1024]
    wpk_r = wpk_t.ap()                                         # [128, 64, 8, 128]
    o_r = o_t.ap()                                             # [128, 64, 2, 512]

    with tile.TileContext(nc) as tc:
        with (
            tc.tile_pool(name="persist", bufs=1) as persist,
            tc.tile_pool(name="wp", bufs=10) as wp,
            tc.tile_pool(name="op", bufs=4) as op,
            tc.tile_pool(name="psp", bufs=1, space="PSUM") as psp,
        ):
            zT = persist.tile([P, KT, T], bf16)       # 16 KiB/partition
            sb_sb = persist.tile([P, T], bf16)
            bT_sb = persist.tile([P, OT], f32)

            w_tiles = {}

            def load_w(ot):
                t = wp.tile([P, KT, P], bf16, tag="w", name=f"w_{ot}")
                nc.sync.dma_start(t, wpk_r[:, ot, :, :])
                w_tiles[ot] = t

            # ---- Phases 1+2: per-expert dispatch and combine ----
            with (
                tc.tile_pool(name="inp", bufs=1) as inp,
                tc.tile_pool(name="xdp", bufs=1) as xdp,
                tc.tile_pool(name="warm", bufs=1) as warm,
            ):
                # -- PE p-state warmup on junk data during DMA cold-start --
                junk = warm.tile([P, P], bf16)
                nc.vector.memset(junk, 0)
                wps = psp.tile([P, 64], f32, tag="ps2", bufs=2, name="wps")
                def junk_mms(n):
                    for i in range(n):
                        nc.tensor.matmul(
                            wps, junk, junk[:, :64],
                            start=(i % 8 == 0), stop=(i % 8 == 7),
                        )

                junk_mms(64)

                # -- input DMA issue, global order --
                xq = {}   # (e, qt) -> [P, 2, 512] tile
                dmq = {}
                c_th = {}

                def load_xdm(e, qt, split):
                    qs = slice(qt * 2, qt * 2 + 2)
                    xq[e, qt] = inp.tile([P, 2, 512], bf16, name=f"x_{e}_{qt}")
                    dmq[e, qt] = inp.tile([P, 2, 512], bf16, name=f"dm_{e}_{qt}")
                    if split:
                        # halves so the first matmuls start on 0.25 MB
                        for hh in range(2):
                            nc.sync.dma_start(
                                dmq[e, qt][:, hh : hh + 1, :],
                                dm_r[:, e, qt * 2 + hh : qt * 2 + hh + 1, :],
                            )
                            nc.sync.dma_start(
                                xq[e, qt][:, hh : hh + 1, :],
                                x_r[:, e, qt * 2 + hh : qt * 2 + hh + 1, :],
                            )
                    else:
                        nc.sync.dma_start(dmq[e, qt], dm_r[:, e, qs, :])
                        nc.sync.dma_start(xq[e, qt], x_r[:, e, qs, :])

                def load_cmb(e):
                    c_th[e] = inp.tile([P, CT, T], bf16, name=f"c_{e}")
                    nc.sync.dma_start(c_th[e], cT_r[:, e, :, :])

                load_xdm(0, 0, True)
                for qt in range(1, 4):
                    load_xdm(0, qt, False)
                load_cmb(0)
                for qt in range(4):
                    load_xdm(1, qt, False)
                load_cmb(1)
                for ot in range(4):
                    load_w(ot)
                nc.sync.dma_start(sb_sb, sb_t.ap())
                nc.sync.dma_start(bT_sb, bT_t.ap())

                # -- phase 1: xd[c, j] = sum_t dm[t, c] * x[t, j] --
                ps1 = {}
                for e in range(EL):
                    ps1[e] = [
                        psp.tile([P, 512], f32, tag="ps1", bufs=4, name=f"ps1_{e}_{ct}")
                        for ct in range(CT)
                    ]
                xd = {}
                for e in range(EL):
                    for tt in range(TT):
                        qt, qi = tt // 2, tt % 2
                        for ct in range(CT):
                            nc.tensor.matmul(
                                ps1[e][ct],
                                dmq[e, qt][:, qi, ct * P : (ct + 1) * P],
                                xq[e, qt][:, qi, :],
                                start=(tt == 0),
                                stop=(tt == TT - 1),
                            )
                        if e == 0 and tt < 3:
                            junk_mms(8)  # keep PE hot through early DMA races
                    xd_e = xdp.tile([P, CT, 512], bf16, name=f"xd_{e}")
                    for ct in range(CT):
                        nc.vector.tensor_copy(xd_e[:, ct, :], ps1[e][ct])
                    xd[e] = xd_e

                # -- phase 2: zT[j, t] = sum_c xd[c, j] * cmbT[c, t] --
                for e in range(EL):
                    for th in range(2):
                        for jt in range(JT):
                            ps2 = psp.tile([P, 512], f32, tag="ps2", bufs=2, name="ps2")
                            for ct in range(CT):
                                nc.tensor.matmul(
                                    ps2,
                                    xd[e][:, ct, jt * P : (jt + 1) * P],
                                    c_th[e][:, ct, th * 512 : (th + 1) * 512],
                                    start=(ct == 0),
                                    stop=(ct == CT - 1),
                                )
                            nc.vector.tensor_copy(
                                zT[:, e * JT + jt, th * 512 : (th + 1) * 512], ps2
                            )

            # ---- Phase 3 (transposed): outT[o,t] = sum_kt w[kt].T @ zT[kt] ----
            if True:
                for ot in range(OT):
                    for pot in range(ot, min(ot + 10, OT)):
                        if pot not in w_tiles:
                            load_w(pot)
                    psum = [
                        psp.tile([P, 512], f32, tag="ps3", bufs=2, name=f"ps3_{ot}_{i}")
                        for i in range(TCH)
                    ]
                    for kt in range(KT):
                        st = w_tiles[ot][:, kt, :]
                        for tch in range(TCH):
                            nc.tensor.matmul(
                                psum[tch],
                                st,
                                zT[:, kt, tch * 512 : (tch + 1) * 512],
                                start=(kt == 0),
                                stop=(kt == KT - 1),
                            )
                    for tch in range(TCH):
                        o_sb = op.tile([P, 512], f16, tag="o_sb")
                        # outT = s_bcast[:, tch] * biasT[:, ot] + psum
                        nc.vector.scalar_tensor_tensor(
                            o_sb,
                            sb_sb[:, tch * 512 : (tch + 1) * 512],
                            bT_sb[:, ot : ot + 1],
                            psum[tch],
                            mybir.AluOpType.mult,
                            mybir.AluOpType.add,
                        )
                        if ot >= OT - 1:
                            nc.sync.dma_start(o_r[:, ot, tch, :], o_sb)
                        else:
                            nc.gpsimd.dma_start(o_r[:, ot, tch, :], o_sb)

    nc.compile()
    return nc


def _get_nc():
    if "nc" not in _CACHE:
        _CACHE["nc"] = _build_nc()
    return _CACHE["nc"]


def _prep_in_maps(x, combine_array, dispatch_mask, weight, bias):
    bf = ml_dtypes.bfloat16
    x = np.asarray(x, dtype=np.float32)
    cmb = np.asarray(combine_array, dtype=np.float32)
    dm = np.asarray(dispatch_mask, dtype=np.float32)
    weight = np.asarray(weight, dtype=np.float32)
    bias = np.asarray(bias, dtype=np.float32)

    # combine transposed+packed to (B, E, CT, P, T) -> [p, e, ct, t]
    cmbT = cmb.transpose(0, 2, 3, 1).reshape(B, E, CT, P, T)
    cmbT = np.ascontiguousarray(cmbT.transpose(0, 3, 1, 2, 4).astype(bf))
    s = cmb.sum(axis=(2, 3), dtype=np.float32)  # (B, T)
    sb = [np.ascontiguousarray(np.broadcast_to(s[b], (P, T)).astype(bf))
          for b in range(B)]
    # wstack[(e,j), o] = w[e, o, j];  w = weight.reshape(E, OUT, IN//E)
    w = weight.reshape(E, OUT, IN // E)
    wstack = np.ascontiguousarray(w.transpose(0, 2, 1)).reshape(IN, OUT)
    # expert-pair h owns wstack rows [h*1024, (h+1)*1024) over the full OUT
    wpk = []
    for h in range(2):
        wh = wstack[h * 1024 : (h + 1) * 1024, :].reshape(KT, P, OT, P)
        wpk.append(np.ascontiguousarray(wh.transpose(1, 2, 0, 3).astype(bf)))
    # bias applied once per pair: even cores get the real bias, odd get zeros
    bT = [
        np.ascontiguousarray(bias.reshape(OT, P).T),
        np.zeros((P, OT), dtype=np.float32),
    ]
    # x packed: [p, e(global), tt, j];  dm packed: [p, e(global), tt, c]
    xb = x.reshape(B, TT, P, E, 512).transpose(0, 2, 3, 1, 4).astype(bf)
    dmb = dm.reshape(B, TT, P, E, C).transpose(0, 2, 3, 1, 4).astype(bf)

    in_maps = []
    for k in range(8):
        b, h = k // 2, k % 2
        in_maps.append(
            {
                "x": np.ascontiguousarray(xb[b][:, 2 * h : 2 * h + 2]),
                "dm": np.ascontiguousarray(dmb[b][:, 2 * h : 2 * h + 2]),
                "cmbT": np.ascontiguousarray(cmbT[b][:, 2 * h : 2 * h + 2]),
                "wpk": wpk[h],
                "sb": sb[b],
                "biasT": bT[h],
            }
        )
    return in_maps


def _enable_persistent_cache():
    try:
        import jax

        jax.config.update("jax_compilation_cache_dir", "/tmp/jax_neff_cache")
        jax.config.update("jax_persistent_cache_min_compile_time_secs", 1.0)
    except Exception:
        pass


def run_spmd(in_maps, trace=False, **kwargs):
    from concourse.bass_utils import run_bass_kernel_spmd

    _enable_persistent_cache()
    nc = _get_nc()
    return run_bass_kernel_spmd(
        nc, in_maps, core_ids=list(range(8)), trace=trace, **kwargs
    )


def kernel(x, combine_array, dispatch_mask, weight, bias, num_experts):
    assert int(num_experts) == E
    in_maps = _prep_in_maps(x, combine_array, dispatch_mask, weight, bias)
    try:
        res = run_spmd(in_maps)
    except Exception:
        # transient device errors (e.g. a wedged core from a prior run)
        # usually clear on retry with a freshly built program
        _CACHE.clear()
        res = run_spmd(in_maps)
    out = np.empty((B, T, OUT), dtype=np.float32)
    for b in range(B):
        pk = (
            np.asarray(res.results[2 * b]["out"], dtype=np.float32)
            + np.asarray(res.results[2 * b + 1]["out"], dtype=np.float32)
        )
        out[b] = pk.transpose(2, 3, 1, 0).reshape(T, OUT)  # (P,OT,TCH,512)->(t,o)
    return out


# revision 12
# speedup vs baseline: 1.1483x; 1.1483x over previous
"""Trainium2 Bass kernel for ExpertsChooseMaskedExpand MoE routing.

Math (reference):
    xd[b,e,c,i] = sum_t x[b,t,(e,i)] * dmask[b,t,e,c]            (dispatch)
    y[b,e,c,o]  = sum_i xd[b,e,c,i] * w[e,o,i] + bias[o]         (expert mm)
    out[b,t,o]  = sum_{e,c} y[b,e,c,o] * cmb[b,t,e,c]            (combine)

Restructured (combine applied before the weight matmul — 155 GF total
instead of 215 GF; the E expert matmuls fuse into one K=2048 matmul):
    xd[b,e][c,j] = sum_t dmask[b,e][t,c] * xr[b,e][t,j]
    zT[b,e][j,t] = sum_c xd[b,e][c,j] * cmbT[b,e][c,t]
    out[b][t,o]  = sum_{(e,j)} zT[b][(e,j),t] * wstack[(e,j),o] + s[b][t]*bias[o]
    where s[b][t] = sum_{e,c} cmb[b,t,e,c],  wstack[(e,j),o] = w[e,o,j]

Sharding: 8 cores = (batch b in 0..3) x (expert pair h in 0..1). Core
(b, h) runs dispatch+combine for experts {2h, 2h+1} only (phases 1-2,
K = 1024 of the fused contraction) and produces a partial output over
the FULL 8192 output columns; the host sums the two partials per batch
(fp32) and unpacks the o-major packing. No on-device collectives. The
bias rank-1 term s[t]*bias[o] is applied only on h=0 cores (h=1 cores
receive a zero biasT), fused into the PSUM->SBUF eviction on the
vector engine.

v2: all-bf16 datapath (bf16 matmuls are ~5% faster than float32r and
halve DMA traffic), fp16 output partials, junk-matmul warmup to ramp
the PE p-state during the DMA cold-start window, dedicated input tiles
(no pool-rotation DMA stalls), input DMA ordered so phase 2 is fed the
moment phase 1 drains.

Phase 3 runs transposed: stationary = weight block (j, o-tile), moving
= zT t-chunks, PSUM holds out^T (o, t).
"""

import numpy as np
import ml_dtypes

B, T, E, C = 4, 1024, 4, 512
IN, OUT = 2048, 8192
P = 128
TT = T // P          # 8  t-tiles
CT = C // P          # 4  c-tiles per expert
JT = 4               # j-tiles per expert (i = 512)
EL = 2               # experts handled per core (expert-pair split)
KT = EL * JT         # 8 k-tiles for the fused matmul (K = 1024 per core)
OT = OUT // P        # 64 o-tiles of 128 (full output width per core)
TCH = 2              # t-chunks of 512

_CACHE = {}


def _build_nc():
    import concourse.mybir as mybir
    import concourse.tile as tile
    from concourse import bacc

    f32 = mybir.dt.float32
    bf16 = mybir.dt.bfloat16
    f16 = mybir.dt.float16

    nc = bacc.Bacc("TRN2", target_bir_lowering=False, debug=False, num_devices=8)
    # x_pk[p, e, tt, j] = x[tt*128+p, e*512+j]
    x_t = nc.dram_tensor("x", (P, EL, TT, 512), bf16, kind="ExternalInput")
    # dm_pk[p, e, tt, c] = dm[tt*128+p, e, c]
    dm_t = nc.dram_tensor("dm", (P, EL, TT, C), bf16, kind="ExternalInput")
    # cT_pk[p, e, ct, t] = cmbT[e, ct*128+p, t]
    cT_t = nc.dram_tensor("cmbT", (P, EL, CT, T), bf16, kind="ExternalInput")
    # wpk[p, ot, kt, oi] = wstack[h*1024 + kt*128+p, ot*128 + oi]
    wpk_t = nc.dram_tensor("wpk", (P, OT, KT, P), bf16, kind="ExternalInput")
    sb_t = nc.dram_tensor("sb", (P, T), bf16, kind="ExternalInput")      # s bcast
    bT_t = nc.dram_tensor("biasT", (P, OT), f32, kind="ExternalInput")
    # out_pk[p, ot, tch, u] = out[tch*512+u, ot*128+p]
    o_t = nc.dram_tensor("out", (P, OT, TCH, 512), f16, kind="ExternalOutput")

    x_r = x_t.ap()                                             # [128, 2, 8, 512]
    dm_r = dm_t.ap()                                           # [128, 2, 8, 512]
    cT_r = cT_t.ap()                                           # [128, 2, 4, 1024]
    wpk_r = wpk_t.ap()                                         # [128, 64, 8, 128]
    o_r = o_t.ap()                                             # [128, 64, 2, 512]

    with tile.TileContext(nc) as tc:
        with (
            tc.tile_pool(name="persist", bufs=1) as persist,
            tc.tile_pool(name="wp", bufs=10) as wp,
            tc.tile_pool(name="op", bufs=4) as op,
        ):
            zT = persist.tile([P, KT, T], bf16)       # 16 KiB/partition
            sb_sb = persist.tile([P, T], bf16)
            bT_sb = persist.tile([P, OT], f32)

            w_tiles = {}

            def load_w(ot):
                t = wp.tile([P, KT, P], bf16, tag="w", name=f"w_{ot}")
                nc.sync.dma_start(t, wpk_r[:, ot, :, :])
                w_tiles[ot] = t

            # ---- Phases 1+2: per-expert dispatch and combine ----
            with (
                tc.tile_pool(name="inp", bufs=1) as inp,
                tc.tile_pool(name="xdp", bufs=1) as xdp,
                tc.tile_pool(name="warm", bufs=1) as warm,
                tc.tile_pool(name="ps_a", bufs=4, space="PSUM") as ps_a,
                tc.tile_pool(name="ps_b", bufs=2, space="PSUM") as ps_b,
            ):
                # -- PE p-state warmup on junk data during DMA cold-start --
                junk = warm.tile([P, P], bf16)
                nc.vector.memset(junk, 0)
                wps = ps_b.tile([P, 64], f32, tag="ps2", name="wps")
                def junk_mms(n):
                    for i in range(n):
                        nc.tensor.matmul(
                            wps, junk, junk[:, :64],
                            start=(i % 8 == 0), stop=(i % 8 == 7),
                        )

                junk_mms(64)

                # -- input DMA issue, global order --
                xq = {}   # (e, qt) -> [P, 2, 512] tile
                dmq = {}
                c_th = {}

                def load_xdm(e, qt, split):
                    qs = slice(qt * 2, qt * 2 + 2)
                    xq[e, qt] = inp.tile([P, 2, 512], bf16, name=f"x_{e}_{qt}")
                    dmq[e, qt] = inp.tile([P, 2, 512], bf16, name=f"dm_{e}_{qt}")
                    if split:
                        # halves so the first matmuls start on 0.25 MB
                        for hh in range(2):
                            nc.sync.dma_start(
                                dmq[e, qt][:, hh : hh + 1, :],
                                dm_r[:, e, qt * 2 + hh : qt * 2 + hh + 1, :],
                            )
                            nc.sync.dma_start(
                                xq[e, qt][:, hh : hh + 1, :],
                                x_r[:, e, qt * 2 + hh : qt * 2 + hh + 1, :],
                            )
                    else:
                        nc.sync.dma_start(dmq[e, qt], dm_r[:, e, qs, :])
                        nc.sync.dma_start(xq[e, qt], x_r[:, e, qs, :])

                def load_cmb(e):
                    c_th[e] = inp.tile([P, CT, T], bf16, name=f"c_{e}")
                    nc.sync.dma_start(c_th[e], cT_r[:, e, :, :])

                load_xdm(0, 0, True)
                for qt in range(1, 4):
                    load_xdm(0, qt, False)
                load_cmb(0)
                for qt in range(4):
                    load_xdm(1, qt, False)
                load_cmb(1)
                for ot in range(4):
                    load_w(ot)
                nc.sync.dma_start(sb_sb, sb_t.ap())
                nc.sync.dma_start(bT_sb, bT_t.ap())

                # -- phase 1: xd[c, j] = sum_t dm[t, c] * x[t, j] --
                ps1 = {}
                for e in range(EL):
                    ps1[e] = [
                        ps_a.tile([P, 512], f32, tag="ps1", name=f"ps1_{e}_{ct}")
                        for ct in range(CT)
                    ]
                xd = {}
                for e in range(EL):
                    for tt in range(TT):
                        qt, qi = tt // 2, tt % 2
                        for ct in range(CT):
                            nc.tensor.matmul(
                                ps1[e][ct],
                                dmq[e, qt][:, qi, ct * P : (ct + 1) * P],
                                xq[e, qt][:, qi, :],
                                start=(tt == 0),
                                stop=(tt == TT - 1),
                            )
                        if e == 0 and tt < 3:
                            junk_mms(8)  # keep PE hot through early DMA races
                    xd_e = xdp.tile([P, CT, 512], bf16, name=f"xd_{e}")
                    for ct in range(CT):
                        nc.vector.tensor_copy(xd_e[:, ct, :], ps1[e][ct])
                    xd[e] = xd_e

                # -- phase 2: zT[j, t] = sum_c xd[c, j] * cmbT[c, t] --
                for e in range(EL):
                    for th in range(2):
                        for jt in range(JT):
                            ps2 = ps_b.tile([P, 512], f32, tag="ps2")
                            for ct in range(CT):
                                nc.tensor.matmul(
                                    ps2,
                                    xd[e][:, ct, jt * P : (jt + 1) * P],
                                    c_th[e][:, ct, th * 512 : (th + 1) * 512],
                                    start=(ct == 0),
                                    stop=(ct == CT - 1),
                                )
                            nc.vector.tensor_copy(
                                zT[:, e * JT + jt, th * 512 : (th + 1) * 512], ps2
                            )

            # ---- Phase 3 (transposed): outT[o,t] = sum_kt w[kt].T @ zT[kt] ----
            with tc.tile_pool(name="ps_c", bufs=8, space="PSUM") as ps_c:
                for ot in range(OT):
                    for pot in range(ot, min(ot + 10, OT)):
                        if pot not in w_tiles:
                            load_w(pot)
                    psum = [
                        ps_c.tile([P, 512], f32, tag="ps3", name=f"ps3_{ot}_{i}")
                        for i in range(TCH)
                    ]
                    for kt in range(KT):
                        st = w_tiles[ot][:, kt, :]
                        for tch in range(TCH):
                            nc.tensor.matmul(
                                psum[tch],
                                st,
                                zT[:, kt, tch * 512 : (tch + 1) * 512],
                                start=(kt == 0),
                                stop=(kt == KT - 1),
                            )
                    for tch in range(TCH):
                        o_sb = op.tile([P, 512], f16, tag="o_sb")
                        # outT = s_bcast[:, tch] * biasT[:, ot] + psum
                        nc.vector.scalar_tensor_tensor(
                            o_sb,
                            sb_sb[:, tch * 512 : (tch + 1) * 512],
                            bT_sb[:, ot : ot + 1],
                            psum[tch],
                            mybir.AluOpType.mult,
                            mybir.AluOpType.add,
                        )
                        if ot >= OT - 1:
                            nc.sync.dma_start(o_r[:, ot, tch, :], o_sb)
                        else:
                            nc.gpsimd.dma_start(o_r[:, ot, tch, :], o_sb)

    nc.compile()
    return nc


def _get_nc():
    if "nc" not in _CACHE:
        _CACHE["nc"] = _build_nc()
    return _CACHE["nc"]


def _prep_in_maps(x, combine_array, dispatch_mask, weight, bias):
    bf = ml_dtypes.bfloat16
    x = np.asarray(x, dtype=np.float32)
    cmb = np.asarray(combine_array, dtype=np.float32)
    dm = np.asarray(dispatch_mask, dtype=np.float32)
    weight = np.asarray(weight, dtype=np.float32)
    bias = np.asarray(bias, dtype=np.float32)

    # combine transposed+packed to (B, E, CT, P, T) -> [p, e, ct, t]
    cmbT = cmb.transpose(0, 2, 3, 1).reshape(B, E, CT, P, T)
    cmbT = np.ascontiguousarray(cmbT.transpose(0, 3, 1, 2, 4).astype(bf))
    s = cmb.sum(axis=(2, 3), dtype=np.float32)  # (B, T)
    sb = [np.ascontiguousarray(np.broadcast_to(s[b], (P, T)).astype(bf))
          for b in range(B)]
    # wstack[(e,j), o] = w[e, o, j];  w = weight.reshape(E, OUT, IN//E)
    w = weight.reshape(E, OUT, IN // E)
    wstack = np.ascontiguousarray(w.transpose(0, 2, 1)).reshape(IN, OUT)
    # expert-pair h owns wstack rows [h*1024, (h+1)*1024) over the full OUT
    wpk = []
    for h in range(2):
        wh = wstack[h * 1024 : (h + 1) * 1024, :].reshape(KT, P, OT, P)
        wpk.append(np.ascontiguousarray(wh.transpose(1, 2, 0, 3).astype(bf)))
    # bias applied once per pair: even cores get the real bias, odd get zeros
    bT = [
        np.ascontiguousarray(bias.reshape(OT, P).T),
        np.zeros((P, OT), dtype=np.float32),
    ]
    # x packed: [p, e(global), tt, j];  dm packed: [p, e(global), tt, c]
    xb = x.reshape(B, TT, P, E, 512).transpose(0, 2, 3, 1, 4).astype(bf)
    dmb = dm.reshape(B, TT, P, E, C).transpose(0, 2, 3, 1, 4).astype(bf)

    in_maps = []
    for k in range(8):
        b, h = k // 2, k % 2
        in_maps.append(
            {
                "x": np.ascontiguousarray(xb[b][:, 2 * h : 2 * h + 2]),
                "dm": np.ascontiguousarray(dmb[b][:, 2 * h : 2 * h + 2]),
                "cmbT": np.ascontiguousarray(cmbT[b][:, 2 * h : 2 * h + 2]),
                "wpk": wpk[h],
                "sb": sb[b],
                "biasT": bT[h],
            }
        )
    return in_maps


def _enable_persistent_cache():
    try:
        import jax

        jax.config.update("jax_compilation_cache_dir", "/tmp/jax_neff_cache")
        jax.config.update("jax_persistent_cache_min_compile_time_secs", 1.0)
    except Exception:
        pass


def run_spmd(in_maps, trace=False, **kwargs):
    from concourse.bass_utils import run_bass_kernel_spmd

    _enable_persistent_cache()
    nc = _get_nc()
    return run_bass_kernel_spmd(
        nc, in_maps, core_ids=list(range(8)), trace=trace, **kwargs
    )


def kernel(x, combine_array, dispatch_mask, weight, bias, num_experts):
    assert int(num_experts) == E
    in_maps = _prep_in_maps(x, combine_array, dispatch_mask, weight, bias)
    try:
        res = run_spmd(in_maps)
    except Exception:
        # transient device errors (e.g. a wedged core from a prior run)
        # usually clear on retry with a freshly built program
        _CACHE.clear()
        res = run_spmd(in_maps)
    out = np.empty((B, T, OUT), dtype=np.float32)
    for b in range(B):
        pk = (
            np.asarray(res.results[2 * b]["out"], dtype=np.float32)
            + np.asarray(res.results[2 * b + 1]["out"], dtype=np.float32)
        )
        out[b] = pk.transpose(2, 3, 1, 0).reshape(T, OUT)  # (P,OT,TCH,512)->(t,o)
    return out


# revision 13
# speedup vs baseline: 1.1754x; 1.0236x over previous
"""Trainium2 Bass kernel for ExpertsChooseMaskedExpand MoE routing.

Math (reference):
    xd[b,e,c,i] = sum_t x[b,t,(e,i)] * dmask[b,t,e,c]            (dispatch)
    y[b,e,c,o]  = sum_i xd[b,e,c,i] * w[e,o,i] + bias[o]         (expert mm)
    out[b,t,o]  = sum_{e,c} y[b,e,c,o] * cmb[b,t,e,c]            (combine)

Restructured (combine applied before the weight matmul — 155 GF total
instead of 215 GF; the E expert matmuls fuse into one K=2048 matmul):
    xd[b,e][c,j] = sum_t dmask[b,e][t,c] * xr[b,e][t,j]
    zT[b,e][j,t] = sum_c xd[b,e][c,j] * cmbT[b,e][c,t]
    out[b][t,o]  = sum_{(e,j)} zT[b][(e,j),t] * wstack[(e,j),o] + s[b][t]*bias[o]
    where s[b][t] = sum_{e,c} cmb[b,t,e,c],  wstack[(e,j),o] = w[e,o,j]

Sharding: 8 cores = (batch b in 0..3) x (expert pair h in 0..1). Core
(b, h) runs dispatch+combine for experts {2h, 2h+1} only (phases 1-2,
K = 1024 of the fused contraction) and produces a partial output over
the FULL 8192 output columns; the host sums the two partials per batch
(fp32) and unpacks the o-major packing. No on-device collectives. The
bias rank-1 term s[t]*bias[o] is applied only on h=0 cores (h=1 cores
receive a zero biasT), fused into the PSUM->SBUF eviction on the
vector engine.

v2: all-bf16 datapath (bf16 matmuls are ~5% faster than float32r and
halve DMA traffic), fp16 output partials, junk-matmul warmup to ramp
the PE p-state during the DMA cold-start window, dedicated input tiles
(no pool-rotation DMA stalls), input DMA ordered so phase 2 is fed the
moment phase 1 drains.

Phase 3 runs transposed: stationary = weight block (j, o-tile), moving
= zT t-chunks, PSUM holds out^T (o, t).
"""

import numpy as np
import ml_dtypes

B, T, E, C = 4, 1024, 4, 512
IN, OUT = 2048, 8192
P = 128
TT = T // P          # 8  t-tiles
CT = C // P          # 4  c-tiles per expert
JT = 4               # j-tiles per expert (i = 512)
EL = 2               # experts handled per core (expert-pair split)
KT = EL * JT         # 8 k-tiles for the fused matmul (K = 1024 per core)
OT = OUT // P        # 64 o-tiles of 128 (full output width per core)
TCH = 2              # t-chunks of 512

_CACHE = {}


def _build_nc():
    import concourse.mybir as mybir
    import concourse.tile as tile
    from concourse import bacc

    f32 = mybir.dt.float32
    bf16 = mybir.dt.bfloat16
    f16 = mybir.dt.float16

    nc = bacc.Bacc("TRN2", target_bir_lowering=False, debug=False, num_devices=8)
    # x_pk[p, e, tt, j] = x[tt*128+p, e*512+j]
    x_t = nc.dram_tensor("x", (P, EL, TT, 512), bf16, kind="ExternalInput")
    # dm_pk[p, e, tt, c] = dm[tt*128+p, e, c]
    dm_t = nc.dram_tensor("dm", (P, EL, TT, C), bf16, kind="ExternalInput")
    # cT_pk[p, e, ct, t] = cmbT[e, ct*128+p, t]
    cT_t = nc.dram_tensor("cmbT", (P, EL, CT, T), bf16, kind="ExternalInput")
    # wpk[p, ot, kt, oi] = wstack[h*1024 + kt*128+p, ot*128 + oi]
    wpk_t = nc.dram_tensor("wpk", (P, OT, KT, P), bf16, kind="ExternalInput")
    sb_t = nc.dram_tensor("sb", (P, T), bf16, kind="ExternalInput")      # s bcast
    bT_t = nc.dram_tensor("biasT", (P, OT), f32, kind="ExternalInput")
    # out_pk[p, ot, tch, u] = out[tch*512+u, ot*128+p]
    o_t = nc.dram_tensor("out", (P, OT, TCH, 512), f16, kind="ExternalOutput")

    x_r = x_t.ap()                                             # [128, 2, 8, 512]
    dm_r = dm_t.ap()                                           # [128, 2, 8, 512]
    cT_r = cT_t.ap()                                           # [128, 2, 4, 1024]
    wpk_r = wpk_t.ap()                                         # [128, 64, 8, 128]
    o_r = o_t.ap()                                             # [128, 64, 2, 512]

    with tile.TileContext(nc) as tc:
        with (
            tc.tile_pool(name="persist", bufs=1) as persist,
            tc.tile_pool(name="wp", bufs=10) as wp,
            tc.tile_pool(name="op", bufs=4) as op,
        ):
            zT = persist.tile([P, KT, T], bf16)       # 16 KiB/partition
            sb_sb = persist.tile([P, T], bf16)
            bT_sb = persist.tile([P, OT], f32)

            w_tiles = {}

            def load_w(ot):
                t = wp.tile([P, KT, P], bf16, tag="w", name=f"w_{ot}")
                nc.sync.dma_start(t, wpk_r[:, ot, :, :])
                w_tiles[ot] = t

            # ---- Phases 1+2: per-expert dispatch and combine ----
            with (
                tc.tile_pool(name="inp", bufs=1) as inp,
                tc.tile_pool(name="xdp", bufs=1) as xdp,
                tc.tile_pool(name="warm", bufs=1) as warm,
                tc.tile_pool(name="ps_a", bufs=4, space="PSUM") as ps_a,
                tc.tile_pool(name="ps_b", bufs=2, space="PSUM") as ps_b,
            ):
                # -- PE p-state warmup on junk data during DMA cold-start --
                junk = warm.tile([P, P], bf16)
                nc.gpsimd.memset(junk, 0)
                wps = ps_b.tile([P, 64], f32, tag="ps2", name="wps")
                def junk_mms(n):
                    for i in range(n):
                        nc.tensor.matmul(
                            wps, junk, junk[:, :64],
                            start=(i % 8 == 0), stop=(i % 8 == 7),
                        )

                junk_mms(64)

                # -- input DMA issue, global order --
                xq = {}   # (e, qt) -> [P, 2, 512] tile
                dmq = {}
                c_th = {}

                def load_xdm(e, qt, split):
                    qs = slice(qt * 2, qt * 2 + 2)
                    xq[e, qt] = inp.tile([P, 2, 512], bf16, name=f"x_{e}_{qt}")
                    dmq[e, qt] = inp.tile([P, 2, 512], bf16, name=f"dm_{e}_{qt}")
                    if split:
                        # halves so the first matmuls start on 0.25 MB
                        for hh in range(2):
                            nc.sync.dma_start(
                                dmq[e, qt][:, hh : hh + 1, :],
                                dm_r[:, e, qt * 2 + hh : qt * 2 + hh + 1, :],
                            )
                            nc.sync.dma_start(
                                xq[e, qt][:, hh : hh + 1, :],
                                x_r[:, e, qt * 2 + hh : qt * 2 + hh + 1, :],
                            )
                    else:
                        nc.sync.dma_start(dmq[e, qt], dm_r[:, e, qs, :])
                        nc.sync.dma_start(xq[e, qt], x_r[:, e, qs, :])

                def load_cmb(e):
                    c_th[e] = inp.tile([P, CT, T], bf16, name=f"c_{e}")
                    nc.sync.dma_start(c_th[e], cT_r[:, e, :, :])

                load_xdm(0, 0, True)
                for qt in range(1, 4):
                    load_xdm(0, qt, False)
                load_cmb(0)
                for qt in range(4):
                    load_xdm(1, qt, False)
                load_cmb(1)
                for ot in range(4):
                    load_w(ot)
                nc.sync.dma_start(sb_sb, sb_t.ap())
                nc.sync.dma_start(bT_sb, bT_t.ap())

                # -- phase 1: xd[c, j] = sum_t dm[t, c] * x[t, j] --
                ps1 = {}
                for e in range(EL):
                    ps1[e] = [
                        ps_a.tile([P, 512], f32, tag="ps1", name=f"ps1_{e}_{ct}")
                        for ct in range(CT)
                    ]
                xd = {}
                for e in range(EL):
                    for tt in range(TT):
                        qt, qi = tt // 2, tt % 2
                        for ct in range(CT):
                            nc.tensor.matmul(
                                ps1[e][ct],
                                dmq[e, qt][:, qi, ct * P : (ct + 1) * P],
                                xq[e, qt][:, qi, :],
                                start=(tt == 0),
                                stop=(tt == TT - 1),
                            )
                        if e == 0 and tt < 3:
                            junk_mms(8)  # keep PE hot through early DMA races
                    xd_e = xdp.tile([P, CT, 512], bf16, name=f"xd_{e}")
                    for ct in range(CT):
                        nc.vector.tensor_copy(xd_e[:, ct, :], ps1[e][ct])
                    xd[e] = xd_e

                # -- phase 2: zT[j, t] = sum_c xd[c, j] * cmbT[c, t] --
                for e in range(EL):
                    for th in range(2):
                        for jt in range(JT):
                            ps2 = ps_b.tile([P, 512], f32, tag="ps2")
                            for ct in range(CT):
                                nc.tensor.matmul(
                                    ps2,
                                    xd[e][:, ct, jt * P : (jt + 1) * P],
                                    c_th[e][:, ct, th * 512 : (th + 1) * 512],
                                    start=(ct == 0),
                                    stop=(ct == CT - 1),
                                )
                            nc.vector.tensor_copy(
                                zT[:, e * JT + jt, th * 512 : (th + 1) * 512], ps2
                            )

            # ---- Phase 3 (transposed): outT[o,t] = sum_kt w[kt].T @ zT[kt] ----
            with tc.tile_pool(name="ps_c", bufs=8, space="PSUM") as ps_c:
                for ot in range(OT):
                    for pot in range(ot, min(ot + 10, OT)):
                        if pot not in w_tiles:
                            load_w(pot)
                    psum = [
                        ps_c.tile([P, 512], f32, tag="ps3", name=f"ps3_{ot}_{i}")
                        for i in range(TCH)
                    ]
                    for kt in range(KT):
                        st = w_tiles[ot][:, kt, :]
                        for tch in range(TCH):
                            nc.tensor.matmul(
                                psum[tch],
                                st,
                                zT[:, kt, tch * 512 : (tch + 1) * 512],
                                start=(kt == 0),
                                stop=(kt == KT - 1),
                            )
                    for tch in range(TCH):
                        o_sb = op.tile([P, 512], f16, tag="o_sb")
                        # outT = s_bcast[:, tch] * biasT[:, ot] + psum
                        nc.vector.scalar_tensor_tensor(
                            o_sb,
                            sb_sb[:, tch * 512 : (tch + 1) * 512],
                            bT_sb[:, ot : ot + 1],
                            psum[tch],
                            mybir.AluOpType.mult,
                            mybir.AluOpType.add,
                        )
                        if ot >= OT - 1:
                            nc.sync.dma_start(o_r[:, ot, tch, :], o_sb)
                        else:
                            nc.gpsimd.dma_start(o_r[:, ot, tch, :], o_sb)

    nc.compile()
    return nc


def _get_nc():
    if "nc" not in _CACHE:
        _CACHE["nc"] = _build_nc()
    return _CACHE["nc"]


def _prep_in_maps(x, combine_array, dispatch_mask, weight, bias):
    bf = ml_dtypes.bfloat16
    x = np.asarray(x, dtype=np.float32)
    cmb = np.asarray(combine_array, dtype=np.float32)
    dm = np.asarray(dispatch_mask, dtype=np.float32)
    weight = np.asarray(weight, dtype=np.float32)
    bias = np.asarray(bias, dtype=np.float32)

    # combine transposed+packed to (B, E, CT, P, T) -> [p, e, ct, t]
    cmbT = cmb.transpose(0, 2, 3, 1).reshape(B, E, CT, P, T)
    cmbT = np.ascontiguousarray(cmbT.transpose(0, 3, 1, 2, 4).astype(bf))
    s = cmb.sum(axis=(2, 3), dtype=np.float32)  # (B, T)
    sb = [np.ascontiguousarray(np.broadcast_to(s[b], (P, T)).astype(bf))
          for b in range(B)]
    # wstack[(e,j), o] = w[e, o, j];  w = weight.reshape(E, OUT, IN//E)
    w = weight.reshape(E, OUT, IN // E)
    wstack = np.ascontiguousarray(w.transpose(0, 2, 1)).reshape(IN, OUT)
    # expert-pair h owns wstack rows [h*1024, (h+1)*1024) over the full OUT
    wpk = []
    for h in range(2):
        wh = wstack[h * 1024 : (h + 1) * 1024, :].reshape(KT, P, OT, P)
        wpk.append(np.ascontiguousarray(wh.transpose(1, 2, 0, 3).astype(bf)))
    # bias applied once per pair: even cores get the real bias, odd get zeros
    bT = [
        np.ascontiguousarray(bias.reshape(OT, P).T),
        np.zeros((P, OT), dtype=np.float32),
    ]
    # x packed: [p, e(global), tt, j];  dm packed: [p, e(global), tt, c]
    xb = x.reshape(B, TT, P, E, 512).transpose(0, 2, 3, 1, 4).astype(bf)
    dmb = dm.reshape(B, TT, P, E, C).transpose(0, 2, 3, 1, 4).astype(bf)

    in_maps = []
    for k in range(8):
        b, h = k // 2, k % 2
        in_maps.append(
            {
                "x": np.ascontiguousarray(xb[b][:, 2 * h : 2 * h + 2]),
                "dm": np.ascontiguousarray(dmb[b][:, 2 * h : 2 * h + 2]),
                "cmbT": np.ascontiguousarray(cmbT[b][:, 2 * h : 2 * h + 2]),
                "wpk": wpk[h],
                "sb": sb[b],
                "biasT": bT[h],
            }
        )
    return in_maps


def _enable_persistent_cache():
    try:
        import jax

        jax.config.update("jax_compilation_cache_dir", "/tmp/jax_neff_cache")
        jax.config.update("jax_persistent_cache_min_compile_time_secs", 1.0)
    except Exception:
        pass


def run_spmd(in_maps, trace=False, **kwargs):
    from concourse.bass_utils import run_bass_kernel_spmd

    _enable_persistent_cache()
    nc = _get_nc()
    return run_bass_kernel_spmd(
        nc, in_maps, core_ids=list(range(8)), trace=trace, **kwargs
    )


def kernel(x, combine_array, dispatch_mask, weight, bias, num_experts):
    assert int(num_experts) == E
    in_maps = _prep_in_maps(x, combine_array, dispatch_mask, weight, bias)
    try:
        res = run_spmd(in_maps)
    except Exception:
        # transient device errors (e.g. a wedged core from a prior run)
        # usually clear on retry with a freshly built program
        _CACHE.clear()
        res = run_spmd(in_maps)
    out = np.empty((B, T, OUT), dtype=np.float32)
    for b in range(B):
        pk = (
            np.asarray(res.results[2 * b]["out"], dtype=np.float32)
            + np.asarray(res.results[2 * b + 1]["out"], dtype=np.float32)
        )
        out[b] = pk.transpose(2, 3, 1, 0).reshape(T, OUT)  # (P,OT,TCH,512)->(t,o)
    return out
